# revision 22
# baseline (speedup 1.0000x reference)
"""Trainium2 Bass kernel for nn_Encoder_82575041233042.

6-layer weight-shared pre-LN transformer encoder, B=2, S=2048, D=1024,
H=16 heads (d_k=64), FF=4096, fp32 I/O, mask all-ones.

Sharding: 8-way row-parallel over the 4096 (batch*seq) token rows; each
core owns 512 contiguous rows of one batch element (cores 0-3 <-> batch
0, cores 4-7 <-> batch 1). Per layer each core computes K/V for its own
rows in fp8-e4m3, AllGathers K/V within its 4-core group, then runs the
layer for its own rows. Output assembled on the host.

v3 changes vs v2:
  - K and V for the whole sequence are loaded into SBUF once per layer
    with a few bulk contiguous DMAs per half (ktall/vfull resident
    tiles); the per-pair kt/vt streaming DMAs, their tiny 64B packets
    and the per-pair vt memsets are gone.  V is staged through a
    [slice][80] layout (ones column baked in at evac time).
  - w1 / wq / wk are pre-transposed on the host so every DMA row is
    1-2KB contiguous (packet count down ~20x).
  - Next layer's LN1 (stats + apply + transposes) is interleaved into
    the tail FFN of the current layer, so the tensor engine does not
    drain at layer boundaries.

Matmuls accumulate in fp32 PSUM.
"""

import sys

if "/opt/trn_rl_repo" not in sys.path:
    sys.path.insert(0, "/opt/trn_rl_repo")

import numpy as np
import ml_dtypes

import bass_rust
import concourse.bass as bass
import concourse.mybir as mybir
import concourse.tile as tile
from concourse.bass_utils import run_bass_kernel_spmd

# ---------------------------------------------------------------------------
# Workaround: this walrus build rejects more than ONE sync wait per
# instruction. Post-pass: split multi-waits onto same-engine NoOps.
# ---------------------------------------------------------------------------

def _split_multiwaits(nc):
    all_created = set()
    for f in nc.m.functions:
        for blk in list(f.blocks):
            insts = [i for i in blk.instructions if i.name not in all_created]
            plans = {}
            for idx, inst in enumerate(insts):
                si = inst.sync_info
                if si is not None and si.on_wait and len(si.on_wait) > 1:
                    waits = list(si.on_wait)
                    nops = []
                    for w in waits[:-1]:
                        nop = nc.engines[inst.engine].nop().ins
                        nop.sync_info = bass_rust.SyncInfo(on_wait=[w], on_update=[])
                        nops.append(nop)
                        all_created.add(nop.name)
                    si.on_wait = waits[-1:]
                    plans[idx] = nops
            if plans:
                new = []
                for idx, inst in enumerate(insts):
                    if idx in plans:
                        new.extend(plans[idx])
                    new.append(inst)
                blk.instructions = new
            else:
                blk.instructions = insts
    for f in nc.m.functions:
        for blk in f.blocks:
            seen = set()
            out = []
            for inst in blk.instructions:
                if inst.name in seen:
                    continue
                seen.add(inst.name)
                out.append(inst)
            blk.instructions = out
    return nc


# ---------------------------------------------------------------------------
B, S, D = 2, 2048, 1024
H, DK, FF = 16, 64, 4096
NL = 6
LN_EPS = 1e-5
NCORES = 8
GROUP = 4                 # cores per batch element
S_OWN = S * B // NCORES   # 512 token rows per core
P = 128
QT = S_OWN // P           # 4 q-tiles of own rows
CH = D // P               # 8 contraction chunks of d_model
FFCH = FF // P            # 32 ff chunks
KTILES = S // P           # 16 key tiles of the full sequence
PAIRS = H // 2            # 8 head pairs
HD = D // 2               # 512
KSEG = P * S_OWN                # one pair-block of the K region
KBYTES = 4 * KSEG               # 4 pairs x [128 i][512] = 262144
SL = 66                         # 64 dims + ones col + 1 pad (t-stride 8*SL %16==0)
VBYTES = P * QT * 8 * SL
BLK = KBYTES + VBYTES

F32 = mybir.dt.float32
BF16 = mybir.dt.bfloat16
FP8 = mybir.dt.float8e4
AF = mybir.ActivationFunctionType
ALU = mybir.AluOpType
DR = mybir.MatmulPerfMode.DoubleRow


def _view(ap, *shape):
    """Reshape a flat DRAM AP."""
    flat = ap
    if len(flat.shape) > 1:
        dims = " ".join(f"a{i}" for i in range(len(flat.shape)))
        flat = flat.rearrange(f"{dims} -> ({dims})")
    names = " ".join(f"b{i}" for i in range(len(shape)))
    kw = {f"b{i}": s for i, s in enumerate(shape)}
    return flat.rearrange(f"({names}) -> {names}", **kw)


def build_program(nl=NL):
    """Build the SPMD Bass program (identical on all 8 cores)."""
    nc = bass.Bass()

    x_own = nc.dram_tensor("x_own", [S_OWN, D], F32, kind="ExternalInput")
    # host pre-arranged weight streams: every DMA row is contiguous
    wqh = nc.dram_tensor("wqh", [PAIRS, P, CH, P], FP8, kind="ExternalInput")
    wkh = nc.dram_tensor("wkh", [PAIRS, P, CH, P], FP8, kind="ExternalInput")
    wv = nc.dram_tensor("wv", [D, D], FP8, kind="ExternalInput")
    wo = nc.dram_tensor("wo", [D, D], BF16, kind="ExternalInput")
    w1h = nc.dram_tensor("w1h", [FFCH, P, CH, P], BF16, kind="ExternalInput")
    w2 = nc.dram_tensor("w2", [FF, D], BF16, kind="ExternalInput")
    e2 = nc.dram_tensor("e2", [DK + 1, P], F32, kind="ExternalInput")
    ident = nc.dram_tensor("ident", [P, P], BF16, kind="ExternalInput")
    out = nc.dram_tensor("out", [S_OWN, D], F32, kind="ExternalOutput")

    # internal combined K+V rings (fp8):
    #   [0:KBYTES]  K as [4 prh][128 i][512]   (i = h2*64 + dk)
    #   [KBYTES:]   V as [128 p][4 t][8 sl][SL] (col 64 = ones)
    kv_own = [[nc.dram_tensor(f"kv_own_{i}_{hh}", [BLK], FP8)
               for hh in range(2)] for i in range(nl)]
    kv_full = [[nc.dram_tensor(f"kv_full_{i}_{hh}", [GROUP, BLK], FP8)
                for hh in range(2)] for i in range(nl)]
    RG = [[0, 1, 2, 3], [4, 5, 6, 7]]

    with tile.TileContext(nc) as tc:
        with (
            tc.tile_pool(name="const", bufs=1) as cpool,
            tc.tile_pool(name="resw", bufs=1) as wpool,      # wv/wo/w2 resident
            tc.tile_pool(name="hpool", bufs=1) as hpool,     # residual h
            tc.tile_pool(name="big", bufs=1) as bpool,       # xnt/qt/o/ht
            tc.tile_pool(name="small", bufs=2) as apool,     # LN scratch
            tc.tile_pool(name="lbuf", bufs=1) as lpool,      # lnf scratch
            tc.tile_pool(name="wqkv", bufs=2) as qkvpool,    # wq/wk pair tiles
            tc.tile_pool(name="wffn", bufs=2) as ffnpool,    # w1 stream
            tc.tile_pool(name="kvs", bufs=2) as kvpool,      # K/V/Q evac
            tc.tile_pool(name="pts", bufs=2) as ptpool,      # P^T tiles
            tc.tile_pool(name="psMM", bufs=2, space="PSUM") as psMM,
            tc.tile_pool(name="psS", bufs=2, space="PSUM") as psS,
            tc.tile_pool(name="psO", bufs=2, space="PSUM") as psO,
        ):
            ident_sb = cpool.tile([P, P], BF16, tag="ident")
            nc.sync.dma_start(ident_sb[:], ident[:])
            warm = psMM.tile([P, P], F32, tag="mm")
            for _ in range(32):
                nc.tensor.matmul(warm[:], ident_sb[:], ident_sb[:],
                                 start=True, stop=True)
            e2_sb = cpool.tile([DK + 1, P], F32, tag="e2")
            nc.sync.dma_start(e2_sb[:], e2[:])

            eps_sb = cpool.tile([P, 1], F32, tag="eps")
            nc.vector.memset(eps_sb[:], LN_EPS)

            h_sb = hpool.tile([P, QT, D], F32, tag="h")
            nc.sync.dma_start(h_sb[:], x_own.rearrange("(t p) d -> p t d", p=P))

            w2r = wpool.tile([P, FFCH, D], BF16, tag="w2r")
            nc.sync.dma_start(w2r[:], w2.rearrange("(f p) n -> p f n", p=P))
            wv_sb = wpool.tile([P, CH, D], FP8, tag="wvr")
            nc.sync.dma_start(wv_sb[:], wv.rearrange("(c p) n -> p c n", p=P))
            wo_sb = wpool.tile([P, CH, D], BF16, tag="wor")
            nc.sync.dma_start(wo_sb[:], wo.rearrange("(c p) n -> p c n", p=P))

            # resident per-layer V for the whole 2048-key sequence.
            # vfull: [128 keys][hh][b][t][sl][SL] with ones at col 64
            vfull = bpool.tile([P, 2, GROUP, QT, 8, SL], FP8, tag="vfull")

            def layernorm_tile(hsl, tagp):
                """(negmu, rstd) [P,1] f32 for one [P, D] row-tile."""
                bns = apool.tile([P, 2, 6], F32, tag=f"{tagp}_bns")
                nc.vector.bn_stats(bns[:, 0, :], hsl[:, 0:HD])
                nc.vector.bn_stats(bns[:, 1, :], hsl[:, HD:D])
                mv = apool.tile([P, 2], F32, tag=f"{tagp}_mv")
                nc.vector.bn_aggr(mv[:], bns[:])
                negmu = apool.tile([P, 1], F32, tag=f"{tagp}_negmu")
                nc.vector.tensor_scalar_mul(negmu[:], mv[:, 0:1], -1.0)
                lnv = apool.tile([P, 1], F32, tag=f"{tagp}_lnv")
                nc.scalar.activation(lnv[:], mv[:, 1:2], AF.Ln, bias=eps_sb[:])
                rstd = apool.tile([P, 1], F32, tag=f"{tagp}_rstd")
                nc.scalar.activation(rstd[:], lnv[:], AF.Exp, scale=-0.5)
                return negmu, rstd

            def ln_transpose_tile(xnt, qt):
                """LN(h row-tile qt) -> xnt[:, :, qt*P:(qt+1)*P]."""
                hsl = h_sb[:, qt, :]
                negmu, rstd = layernorm_tile(hsl, "ln")
                xb = apool.tile([P, D], BF16, tag="xn_blk", bufs=1)
                nc.vector.tensor_scalar(
                    xb[:], hsl, negmu[:], rstd[:], ALU.add, ALU.mult,
                )
                for c in range(CH):
                    pst = psMM.tile([P, P], BF16, tag="mm")
                    nc.tensor.transpose(pst[:], xb[:, c * P:(c + 1) * P],
                                        ident_sb[:])
                    nc.vector.tensor_copy(xnt[:, c, qt * P:(qt + 1) * P],
                                          pst[:])

            out_v = out.rearrange("(t p) d -> p t d", p=P)

            def lnf_tile(qt):
                hsl = h_sb[:, qt, :]
                negmu, rstd = layernorm_tile(hsl, "lnf")
                for hf in range(2):
                    ot = lpool.tile([P, HD], F32, tag="lnf_out", bufs=1)
                    nc.vector.tensor_scalar(
                        ot[:], hsl[:, hf * HD:(hf + 1) * HD],
                        negmu[:], rstd[:], ALU.add, ALU.mult
                    )
                    nc.sync.dma_start(
                        out_v[:, qt, hf * HD:(hf + 1) * HD], ot[:])

            xnt1 = bpool.tile([P, CH, S_OWN], FP8, tag="xnt", bufs=1,
                              name="xnt1_0")

            for L in range(nl):
                if L == 0:
                    with nc.named_scope("L0_ln1"):
                        for qt in range(QT):
                            ln_transpose_tile(xnt1, qt)
                # for L>0 xnt1 was filled during layer L-1's tail

                xnt1_l = xnt1

                def k_pair(pr):
                    hh, prh = divmod(pr, PAIRS // 2)
                    wkc = qkvpool.tile([P, CH, P], FP8, tag="wqkc",
                                       name=f"wkc{pr}")
                    nc.sync.dma_start(wkc[:], wkh[pr])
                    psk = psMM.tile([P, S_OWN], F32, tag="mm", name=f"psk{pr}")
                    for c in range(CH // 2):
                        nc.tensor.matmul(
                            psk[:], wkc[:, 2 * c:2 * c + 2, :],
                            xnt1_l[:, 2 * c:2 * c + 2, :],
                            start=(c == 0), stop=(c == CH // 2 - 1),
                            perf_mode=DR,
                        )
                    ktev = kvpool.tile([P, S_OWN], FP8, tag="ktev",
                                       name=f"ktev{pr}")
                    nc.scalar.copy(ktev[:], psk[:])
                    kreg = _view(kv_own[L][hh][0:KBYTES], 4, P, S_OWN)[prh]
                    nc.sync.dma_start(kreg[:], ktev[:])

                def v_half(hf):
                    vreg = _view(kv_own[L][hf][KBYTES:BLK], P, QT, 8, SL)
                    for t in range(QT):
                        psv = psMM.tile([P, HD], F32, tag="mm", name=f"psv{t}")
                        for c in range(CH // 2):
                            nc.tensor.matmul(
                                psv[:],
                                xnt1_l[:, 2 * c:2 * c + 2, t * P:(t + 1) * P],
                                wv_sb[:, 2 * c:2 * c + 2, hf * HD:(hf + 1) * HD],
                                start=(c == 0), stop=(c == CH // 2 - 1),
                                perf_mode=DR,
                            )
                        vev = kvpool.tile([P, 8, SL], FP8, tag="vev",
                                          name=f"vev{t}")
                        nc.vector.memset(vev[:, :, DK:SL], 0.0)
                        nc.vector.memset(vev[:, :, DK:DK + 1], 1.0)
                        nc.vector.tensor_copy(
                            vev[:, :, 0:DK],
                            psv[:].rearrange("p (s d) -> p s d", s=8))
                        nc.sync.dma_start(vreg[:, t, :, :], vev[:])

                def q_pair(pr):
                    wqc = qkvpool.tile([P, CH, P], FP8, tag="wqkc")
                    nc.sync.dma_start(wqc[:], wqh[pr])
                    psq = psMM.tile([P, S_OWN], F32, tag="mm")
                    for c in range(CH // 2):
                        nc.tensor.matmul(
                            psq[:], wqc[:, 2 * c:2 * c + 2, :],
                            xnt1_l[:, 2 * c:2 * c + 2, :],
                            start=(c == 0), stop=(c == CH // 2 - 1),
                            perf_mode=DR,
                        )
                    nc.scalar.copy(qt_sb[:, pr, :], psq[:])

                def load_kv(hh):
                    """Bulk-load gathered V of half hh into SBUF, per b
                    block so PV can start as soon as its block lands."""
                    for b in range(GROUP):
                        vsrc = kv_full[L][hh][b:b + 1, KBYTES:BLK]
                        vsrc = vsrc.rearrange("b (p c) -> (b p) c", p=P)
                        nc.sync.dma_start(
                            vfull[:, hh, b, :, :, :]
                            .rearrange("p t s c -> p (t s c)"),
                            vsrc)

                with nc.named_scope(f"L{L}_kv"):
                    for pr in range(PAIRS // 2):
                        k_pair(pr)
                    v_half(0)
                    nc.gpsimd.collective_compute(
                        "AllGather", ALU.bypass, replica_groups=RG,
                        ins=[kv_own[L][0][:]], outs=[kv_full[L][0][:]],
                    )

                with nc.named_scope(f"L{L}_q"):
                    qt_sb = bpool.tile([P, PAIRS, S_OWN], FP8, tag="qt_sb",
                                       name=f"qt_{L}")
                    # fill the gather0 window: second-half K/V, all Q
                    for pr in range(PAIRS // 2, PAIRS):
                        k_pair(pr)
                    v_half(1)
                    nc.gpsimd.collective_compute(
                        "AllGather", ALU.bypass, replica_groups=RG,
                        ins=[kv_own[L][1][:]], outs=[kv_full[L][1][:]],
                    )
                    for pr in range(PAIRS):
                        q_pair(pr)
                    load_kv(0)

                # ---- attention + rest, 2 token waves, interleaved ----------
                WAVES = 2
                WQ = S_OWN // WAVES
                NG = KTILES // 2
                o_sb = bpool.tile([P, PAIRS, S_OWN], BF16, tag="o_sb")
                xnt2 = bpool.tile([P, CH, S_OWN], BF16, tag="xnt2",
                                  name=f"xnt2_{L}")
                ht = [bpool.tile([P, FFCH, WQ], BF16, tag="ht_sb", bufs=2,
                                 name=f"ht{L}_{w}") for w in range(WAVES)]
                l2s = {}

                def attn_pair(w, pr):
                    hh, prh = divmod(pr, PAIRS // 2)
                    qlo, qhi = w * WQ, (w + 1) * WQ
                    kt = kvpool.tile([P, GROUP, S_OWN], FP8, tag="kt",
                                     name=f"kt{w}_{pr}")
                    kseg = kv_full[L][hh][:, prh * KSEG:(prh + 1) * KSEG]
                    nc.sync.dma_start(
                        kt[:], kseg.rearrange("b (p s) -> p b s", p=P))
                    pso = [psO.tile([DK + 1, WQ], F32, tag="oo",
                                    name=f"pso{w}_{pr}_{h2}")
                           for h2 in range(2)]
                    for g in range(NG):
                        pss = psS.tile([P, 2, 2, WQ], F32, tag="ss",
                                       name="pss")
                        for u in range(2):
                            j = 2 * g + u
                            b, jj = divmod(j, GROUP)
                            for h2 in range(2):
                                lo = h2 * DK
                                nc.tensor.matmul(
                                    pss[:, h2, u, :],
                                    kt[lo:lo + DK, b, jj * P:(jj + 1) * P],
                                    qt_sb[lo:lo + DK, pr, qlo:qhi],
                                    start=True, stop=True,
                                )
                        pt = ptpool.tile([P, 2, 2, WQ], FP8, tag="pt")
                        nc.scalar.activation(pt[:], pss[:], AF.Exp,
                                             scale=0.125)
                        b = (2 * g) // GROUP
                        tp = (2 * g) % GROUP
                        for h2 in range(2):
                            sl = prh * 2 + h2
                            nc.tensor.matmul(
                                pso[h2][:],
                                vfull[:, hh, b, tp:tp + 2, sl, 0:DK + 1],
                                pt[:, h2, :, :],
                                start=(g == 0), stop=(g == NG - 1),
                                perf_mode=DR,
                            )
                    if pr % 2 == 0:
                        l2p = apool.tile([DK + 1, 2, WQ], F32, tag="l2p",
                                         name=f"l2p{w}_{pr}", bufs=4)
                        nc.vector.memset(l2p[:], 1.0)
                        l2s[(w, pr)] = l2p
                    else:
                        l2p = l2s[(w, pr - 1)]
                    for h2 in range(2):
                        nc.vector.tensor_copy(
                            l2p[h2 * DK:h2 * DK + 1, pr % 2, :],
                            pso[h2][DK:DK + 1, :])
                        nc.vector.tensor_copy(
                            o_sb[h2 * DK:(h2 + 1) * DK, pr, qlo:qhi],
                            pso[h2][0:DK, :])

                def norm_oproj(w):
                    qlo, qhi = w * WQ, (w + 1) * WQ
                    for pr2 in range(PAIRS // 2):
                        l2p = l2s[(w, 2 * pr2)]
                        lnl = apool.tile([DK + 1, 2, WQ], F32, tag="lnl",
                                         bufs=1)
                        nc.scalar.activation(lnl[:], l2p[:], AF.Ln)
                        nc.scalar.activation(lnl[:], lnl[:], AF.Exp,
                                             scale=-1.0)
                        psl = psMM.tile([P, 2, WQ], F32, tag="mm", name="psl")
                        nc.tensor.matmul(psl[:], e2_sb[:], lnl[:],
                                         start=True, stop=True)
                        ob = o_sb[:, 2 * pr2:2 * pr2 + 2, qlo:qhi]
                        nc.vector.tensor_mul(ob, ob, psl[:])
                    for qt in range(2 * w, 2 * w + 2):
                        psa = [psMM.tile([P, HD], F32, tag="mm",
                                         name=f"psa{hf}")
                               for hf in range(2)]
                        for pr in range(PAIRS):
                            for hf in range(2):
                                nc.tensor.matmul(
                                    psa[hf][:],
                                    o_sb[:, pr, qt * P:(qt + 1) * P],
                                    wo_sb[:, pr, hf * HD:(hf + 1) * HD],
                                    start=(pr == 0), stop=(pr == PAIRS - 1),
                                )
                        for hf in range(2):
                            hsl = h_sb[:, qt, hf * HD:(hf + 1) * HD]
                            nc.vector.tensor_add(hsl, hsl, psa[hf][:])

                def ln2_wave(w):
                    for qt in range(2 * w, 2 * w + 2):
                        hsl = h_sb[:, qt, :]
                        negmu, rstd = layernorm_tile(hsl, "ln")
                        xb = apool.tile([P, D], BF16, tag="xn_blk", bufs=1)
                        nc.vector.tensor_scalar(
                            xb[:], hsl, negmu[:], rstd[:], ALU.add, ALU.mult,
                        )
                        nc.sync.dma_start_transpose(
                            xnt2[:, :, qt * P:(qt + 1) * P], xb[:])

                def ffn1_chunk(w, f):
                    qlo, qhi = w * WQ, (w + 1) * WQ
                    w1c = ffnpool.tile([P, CH, P], BF16, tag="w1c")
                    nc.sync.dma_start(w1c[:], w1h[f])
                    psh = psMM.tile([P, WQ], F32, tag="mm", name="psh")
                    for c in range(CH):
                        nc.tensor.matmul(
                            psh[:], w1c[:, c, :], xnt2[:, c, qlo:qhi],
                            start=(c == 0), stop=(c == CH - 1),
                        )
                    nc.vector.tensor_scalar_max(ht[w][:, f, :], psh[:], 0.0)

                def ffn2_qt(w, qt):
                    qv = qt - 2 * w
                    psf = [psMM.tile([P, HD], F32, tag="mm", name=f"psf{hf}")
                           for hf in range(2)]
                    for f in range(FFCH):
                        for hf in range(2):
                            nc.tensor.matmul(
                                psf[hf][:],
                                ht[w][:, f, qv * P:(qv + 1) * P],
                                w2r[:, f, hf * HD:(hf + 1) * HD],
                                start=(f == 0), stop=(f == FFCH - 1),
                            )
                    for hf in range(2):
                        hsl = h_sb[:, qt, hf * HD:(hf + 1) * HD]
                        nc.vector.tensor_add(hsl, hsl, psf[hf][:])

                with nc.named_scope(f"L{L}_attn0"):
                    for pr in range(PAIRS // 2):
                        attn_pair(0, pr)
                        if pr == PAIRS // 2 - 1:
                            load_kv(1)
                    for pr in range(PAIRS // 2, PAIRS):
                        attn_pair(0, pr)

                with nc.named_scope(f"L{L}_mid"):
                    units = [lambda: norm_oproj(0), lambda: ln2_wave(0)]
                    for f in range(FFCH):
                        units.append(lambda f=f: ffn1_chunk(0, f))
                    units.append(lambda: ffn2_qt(0, 0))
                    units.append(lambda: ffn2_qt(0, 1))
                    ui = 0
                    for pr in range(PAIRS):
                        attn_pair(1, pr)
                        upto = (len(units) * (pr + 1)) // PAIRS
                        while ui < upto:
                            units[ui]()
                            ui += 1

                with nc.named_scope(f"L{L}_rest1"):
                    norm_oproj(1)
                    ln2_wave(1)
                    # interleave next layer's LN1 (or the final LN) into
                    # the tail FFN
                    if L + 1 < nl:
                        nxt = bpool.tile([P, CH, S_OWN], FP8, tag="xnt",
                                         bufs=1, name=f"xnt1_{L + 1}")
                        tail = [lambda: ln_transpose_tile(nxt, 0),
                                lambda: ln_transpose_tile(nxt, 1)]
                        late = [lambda: ln_transpose_tile(nxt, 2),
                                lambda: ln_transpose_tile(nxt, 3)]
                    else:
                        nxt = None
                        tail = [lambda: lnf_tile(0), lambda: lnf_tile(1)]
                        late = [lambda: lnf_tile(2), lambda: lnf_tile(3)]
                    ti = 0
                    for f in range(FFCH):
                        ffn1_chunk(1, f)
                        upto = (len(tail) * (f + 1)) // FFCH
                        while ti < upto:
                            tail[ti]()
                            ti += 1
                    ffn2_qt(1, 2)
                    late[0]()
                    ffn2_qt(1, 3)
                    late[1]()
                    if nxt is not None:
                        xnt1 = nxt


    _split_multiwaits(nc)
    return nc


_CACHED = {}


def _get_program():
    if "nc" not in _CACHED:
        _CACHED["nc"] = build_program()
    return _CACHED["nc"]


def make_in_maps(inputs):
    x = np.asarray(inputs["x"], np.float32)
    bf = ml_dtypes.bfloat16
    f8 = ml_dtypes.float8_e4m3
    w1 = np.asarray(inputs["w1"], np.float32).astype(bf)
    # [f][p][c][j] = w1[c*128+p, f*128+j] : contiguous 2KB DMA rows
    w1hm = np.ascontiguousarray(
        w1.reshape(CH, P, FFCH, P).transpose(2, 1, 0, 3)
    )
    wqm = np.asarray(inputs["wq"], np.float32).astype(f8)
    wkm = np.asarray(inputs["wk"], np.float32).astype(f8)
    # [pr][p][c][j] = w[c*128+p, pr*128+j]
    wqhm = np.ascontiguousarray(wqm.reshape(CH, P, PAIRS, P).transpose(2, 1, 0, 3))
    wkhm = np.ascontiguousarray(wkm.reshape(CH, P, PAIRS, P).transpose(2, 1, 0, 3))
    e2m = np.zeros((DK + 1, P), np.float32)
    e2m[0, 0:DK] = 1.0
    e2m[DK, DK:P] = 1.0
    common = {
        "wqh": wqhm,
        "wkh": wkhm,
        "wv": np.asarray(inputs["wv"], np.float32).astype(f8),
        "wo": np.asarray(inputs["wo"], np.float32).astype(bf),
        "w1h": w1hm,
        "w2": np.asarray(inputs["w2"], np.float32).astype(bf),
        "e2": e2m,
        "ident": np.eye(P, dtype=bf),
    }
    xr = x.reshape(B * S, D)
    in_maps = []
    for c in range(NCORES):
        m = dict(common)
        m["x_own"] = np.ascontiguousarray(xr[c * S_OWN:(c + 1) * S_OWN])
        in_maps.append(m)
    return in_maps


def kernel(**inputs):
    in_maps = make_in_maps(inputs)
    nc = _get_program()
    res = run_bass_kernel_spmd(nc, in_maps, list(range(NCORES)))
    full = np.concatenate([res.results[c]["out"] for c in range(NCORES)], axis=0)
    return full.reshape(B, S, D).astype(np.float32)
